# revision 41
# baseline (speedup 1.0000x reference)
"""Distributed multi-head attention kernel for one TRN2 chip (8 NeuronCores).

Problem: b=2, n=2048, dim=1024, heads=16, hd=64.
  qkv = x @ Wqkv.T  (qkv-major split) -> RoPE(q,k) -> softmax(q k^T/8) v
  -> merge heads -> @ Wproj.T + bproj

Sharding: each core owns 2 heads (of 16) for BOTH batches. QKV projection,
RoPE and attention are fully head-local. Two 8-way AllToAlls (one per batch,
256-token slices per core) redistribute attention outputs head-major ->
token-major; batch 0's A2A + projection hide under batch 1's attention.
Core c outputs tokens [256c:256c+256] of each batch; host reassembles.

Per-core inputs (see make_in_maps) are pre-transposed/pre-cast on the host so
no DMA-xbar transposes are needed (Tile serializes those globally):
  x        [1024, 4096] bf16  x^T: channels x flat tokens
  wqkv     [1024, 384]  bf16  (q|k|v rows for my heads)^T
  wproj    [1024, 1024] bf16  Wproj^T: [d', f]
  bproj    [1, 1024]    f32
  sin/cos  [2048, 64]   f32
  ident    [128, 128]   bf16  identity for PE transposes
  out      [512, 1024]  f32   rows 0:256 = b0 slice, 256:512 = b1 slice

All matmuls bf16 (PSUM accumulates f32). scoresT layout [k_j, q_i] (k
stationary, both heads row-packed across the 128 partitions) so softmax needs
no transposes: denominators come from a ones-column appended to v. exp on
ScalarE with fused 1/8 scale; no max subtraction (scores std ~2). The QKV
chain for each batch is software-pipelined into that batch's first
attention q-block (lag 4 tiles) so PE/ACT streams interleave.
"""

import os
import numpy as np

NUM_CORES = 8
B, N, DIM, NH, HD = 2, 2048, 1024, 16, 64
T = B * N                 # 4096 flat tokens
HPC = NH // NUM_CORES     # 2 heads per core
P = 128
CT = DIM // P             # 8 channel tiles
SL = N // NUM_CORES       # 256 output tokens per core per batch
QW = HPC * HD             # 128
FQKV = 3 * QW             # 384
QB = 512                  # attention q-block width
TTH = N // P              # 16 token tiles per batch

_CACHE = {}


def _build_nc():
    from concourse import bacc, mybir, tile

    f32 = mybir.dt.float32
    bf16 = mybir.dt.bfloat16
    Exp = mybir.ActivationFunctionType.Exp
    mult = mybir.AluOpType.mult
    add = mybir.AluOpType.add

    nc = bacc.Bacc("TRN2", target_bir_lowering=False, debug=False,
                   num_devices=NUM_CORES)

    x_d = nc.dram_tensor("x", [DIM, T], bf16, kind="ExternalInput")
    wqkv_d = nc.dram_tensor("wqkv", [DIM, FQKV], bf16, kind="ExternalInput")
    wproj_d = nc.dram_tensor("wproj", [DIM, DIM], bf16, kind="ExternalInput")
    bproj_d = nc.dram_tensor("bproj", [1, DIM], f32, kind="ExternalInput")
    sin_d = nc.dram_tensor("sin", [N, HD], f32, kind="ExternalInput")
    cos_d = nc.dram_tensor("cos", [N, HD], f32, kind="ExternalInput")
    ident_d = nc.dram_tensor("ident", [P, P], bf16, kind="ExternalInput")
    out_d = nc.dram_tensor("out", [2 * SL, DIM], f32, kind="ExternalOutput")
    a2a_in = [nc.dram_tensor(f"a2a_in{b}", [NUM_CORES * P, SL], bf16)
              for b in range(B)]
    a2a_out = [nc.dram_tensor(f"a2a_out{b}", [NUM_CORES * P, SL], bf16)
               for b in range(B)]

    with tile.TileContext(nc) as tc:
        with (
            tc.tile_pool(name="persist", bufs=1) as pers,
            tc.tile_pool(name="work", bufs=4) as wp,
            tc.tile_pool(name="expp", bufs=4) as ep,
            tc.tile_pool(name="psA", bufs=2, space="PSUM") as psA,   # qkv/bc/proj/tp
            tc.tile_pool(name="psS", bufs=2, space="PSUM") as psS,   # scores
            tc.tile_pool(name="psV", bufs=1, space="PSUM") as psV,   # av accum
        ):
            # ---------------- persistent SBUF ----------------
            wqkvT = pers.tile([P, CT * FQKV], bf16)     # ct-block: [128c, 384f]
            wprojT = pers.tile([P, CT * DIM], bf16)     # dt-block: [128d', 1024f]
            xT = pers.tile([P, CT * T], bf16)           # ct-block: [128c, 4096t]
            qT = pers.tile([P, T], bf16)                # [d(2 heads), flat t]
            kT = pers.tile([P, T], bf16)
            v_sb = pers.tile([P, HPC * (T // P) * 65], bf16)
            aoT = pers.tile([P, T], bf16)               # [d', flat t]
            aoTr = pers.tile([P, B * NUM_CORES * SL], bf16)  # per b: [d'chnk, 256t]
            sin4 = pers.tile([P, 16 * 4 * HD], bf16)
            cos4 = pers.tile([P, 16 * 4 * HD], bf16)
            sneg4 = pers.tile([P, 16 * 4 * HD], bf16)
            ones_col = pers.tile([1, P], bf16)
            bias_bf = pers.tile([1, DIM], bf16)
            ident = pers.tile([P, P], bf16)

            nc.vector.memset(ones_col, 1.0)
            nc.vector.memset(v_sb, 1.0)                 # ones cols survive

            # ---------------- prep loads (no xbar transposes) ----------------
            # scalar queue: ident, sincos, wqkv, bias (in need order);
            # sync queue: xT, first token-quarter in half-column passes so
            # tile-0's QKV matmuls start as early as possible
            nc.scalar.dma_start(ident, ident_d.ap())
            for ct in range(CT):
                nc.scalar.dma_start(wqkvT[:, FQKV * ct:FQKV * (ct + 1)],
                                    wqkv_d[P * ct:P * (ct + 1), :])
            sin_f = wp.tile([P, 16 * HD], f32, tag="scload", bufs=1)
            cos_f = wp.tile([P, 16 * HD], f32, tag="scload2", bufs=1)
            nc.scalar.dma_start(
                sin_f.rearrange("p (pt d) -> p pt d", pt=16),
                sin_d.ap().rearrange("(pt p) d -> p pt d", p=P))
            nc.scalar.dma_start(
                cos_f.rearrange("p (pt d) -> p pt d", pt=16),
                cos_d.ap().rearrange("(pt p) d -> p pt d", p=P))
            for half in range(2):
                for ct in range(CT):
                    nc.sync.dma_start(
                        xT[:, T * ct + 512 * half:T * ct + 512 * (half + 1)],
                        x_d[P * ct:P * (ct + 1), 512 * half:512 * (half + 1)])
            for tq in range(1, 4):
                for ct in range(CT):
                    nc.sync.dma_start(
                        xT[:, T * ct + 1024 * tq:T * ct + 1024 * (tq + 1)],
                        x_d[P * ct:P * (ct + 1), 1024 * tq:1024 * (tq + 1)])

            bt = wp.tile([1, DIM], f32, tag="bload", bufs=1)
            nc.scalar.dma_start(bt, bproj_d[:, :])
            nc.vector.tensor_copy(bias_bf, bt)
            s4 = sin4.rearrange("p (pt c d) -> p pt c d", pt=16, c=4)
            c4 = cos4.rearrange("p (pt c d) -> p pt c d", pt=16, c=4)
            n4 = sneg4.rearrange("p (pt c d) -> p pt c d", pt=16, c=4)
            sf = sin_f.rearrange("p (pt d) -> p pt d", pt=16)
            cf = cos_f.rearrange("p (pt d) -> p pt d", pt=16)
            for c in range(4):
                nc.vector.tensor_copy(s4[:, :, c, :], sf)
                nc.vector.tensor_copy(c4[:, :, c, :], cf)
            nc.vector.tensor_scalar_mul(n4[:, :, :, 0:32], s4[:, :, :, 0:32], -1.0)
            nc.vector.tensor_copy(n4[:, :, :, 32:64], s4[:, :, :, 32:64])

            def emit_qkv_tile(b, tt, act_copies=False):
                """QKV matmul + RoPE + PE transposes for one 128-token tile.

                act_copies: route PSUM->SBUF copies to ScalarE (only safe in
                windows where the exp stream has slack, i.e. b0's first
                q-block)."""
                cp = nc.scalar.copy if act_copies else nc.vector.tensor_copy
                ftt = TTH * b + tt
                qkvp = psA.tile([P, 512], f32, tag="mm", name="qkvp")
                for ct in range(CT):
                    base = T * ct + N * b
                    nc.tensor.matmul(
                        qkvp[:, 0:FQKV],
                        xT[:, base + P * tt:base + P * (tt + 1)],
                        wqkvT[:, FQKV * ct:FQKV * (ct + 1)],
                        start=(ct == 0), stop=(ct == CT - 1))
                qkc = wp.tile([P, 2 * QW], bf16, tag="qkc")
                cp(qkc, qkvp[:, 0:2 * QW])
                pt = tt % 16
                qk3 = qkc.rearrange("p (c d) -> p c d", c=4)
                t1 = wp.tile([P, 2 * QW], bf16, tag="t1")
                t13 = t1.rearrange("p (c d) -> p c d", c=4)
                nc.vector.tensor_tensor(t13[:, :, 0:32], qk3[:, :, 32:64],
                                        n4[:, pt, :, 0:32], mult)
                nc.vector.tensor_tensor(t13[:, :, 32:64], qk3[:, :, 0:32],
                                        n4[:, pt, :, 32:64], mult)
                qkcos = wp.tile([P, 2 * QW], bf16, tag="qkcos")
                nc.vector.tensor_tensor(
                    qkcos, qkc, cos4[:, 4 * HD * pt:4 * HD * (pt + 1)], mult)
                qrope = wp.tile([P, QW], bf16, tag="qrope")
                krope = wp.tile([P, QW], bf16, tag="krope")
                nc.vector.tensor_tensor(qrope, qkcos[:, 0:QW], t1[:, 0:QW], add)
                nc.vector.tensor_tensor(krope, qkcos[:, QW:2 * QW],
                                        t1[:, QW:2 * QW], add)
                tp = psA.tile([P, 2 * P], bf16, tag="mm", name="tp")
                nc.tensor.transpose(tp[:, 0:P], qrope, ident)
                nc.tensor.transpose(tp[:, P:2 * P], krope, ident)
                cp(qT[:, P * ftt:P * (ftt + 1)], tp[:, 0:P])
                cp(kT[:, P * ftt:P * (ftt + 1)], tp[:, P:2 * P])
                vv = v_sb.rearrange("p (h t e) -> p h t e", h=HPC, t=T // P)
                nc.vector.tensor_copy(
                    vv[:, :, ftt, 0:HD],
                    qkvp[:, 2 * QW:3 * QW].rearrange("p (h d) -> p h d", h=HPC))

            def emit_scores_exp(b, qq, jt):
                """Scores + exp for one (q-block, j-tile); returns the exp tile."""
                ftt = TTH * b + jt
                sp = psS.tile([P, HPC * QB], f32, tag="scores", name="sp")
                for h in range(HPC):
                    nc.tensor.matmul(
                        sp[:, QB * h:QB * (h + 1)],
                        kT[HD * h:HD * (h + 1), P * ftt:P * (ftt + 1)],
                        qT[HD * h:HD * (h + 1),
                           N * b + QB * qq:N * b + QB * (qq + 1)],
                        start=True, stop=True)
                et = ep.tile([P, HPC * QB], bf16, tag="expT", name="et")
                nc.scalar.activation(et, sp, Exp, scale=float(HD) ** -0.5)
                return et

            def emit_av(b, qq, jt, av, et):
                ftt = TTH * b + jt
                for h in range(HPC):
                    blk = (h * (T // P) + ftt) * 65
                    nc.tensor.matmul(av[h], v_sb[:, blk:blk + 65],
                                     et[:, QB * h:QB * (h + 1)],
                                     start=(jt == 0), stop=(jt == TTH - 1))

            def emit_avf(avp):
                avf = wp.tile([65, HPC * QB], f32, tag="avf", bufs=2, name="avf")
                nc.vector.tensor_copy(avf, avp)
                return avf

            def emit_norm_rest(b, qq, avf):
                """Denominator broadcast + reciprocal + normalize, plus the A2A
                staging of this q-block's two 256-token chunks."""
                for h in range(HPC):
                    sums = wp.tile([1, QB], bf16, tag="sums", name="sums")
                    nc.vector.tensor_copy(sums, avf[64:65, QB * h:QB * (h + 1)])
                    bc = psA.tile([64, QB], f32, tag="mm", name="bc")
                    nc.tensor.matmul(bc, ones_col[:, 0:64], sums,
                                     start=True, stop=True)
                    rc = wp.tile([64, QB], f32, tag="recip", bufs=2, name="rc")
                    nc.vector.reciprocal_approx_fast(rc, bc)
                    nc.vector.tensor_tensor(
                        aoT[HD * h:HD * (h + 1),
                            N * b + QB * qq:N * b + QB * (qq + 1)],
                        avf[0:64, QB * h:QB * (h + 1)], rc, mult)
                # stage chunks 2qq, 2qq+1 of this batch's A2A input
                r0 = 2 * P * qq
                a2i = a2a_in[b][r0:r0 + 2 * P].rearrange("(c p) t -> p c t", p=P)
                nc.sync.dma_start(
                    a2i, aoT[:, N * b + QB * qq:N * b + QB * (qq + 1)].rearrange(
                        "p (c t) -> p c t", c=2))

            def emit_a2a(b):
                nc.gpsimd.collective_compute(
                    "AllToAll", mybir.AluOpType.bypass,
                    replica_groups=[list(range(NUM_CORES))],
                    ins=[a2a_in[b].ap().opt()], outs=[a2a_out[b].ap().opt()])

            def emit_proj(b):
                """Fetch A2A result for batch b and project my 256-token slice."""
                rbase = NUM_CORES * SL * b
                a2o = a2a_out[b].ap().rearrange("(c p) t -> p c t", p=P)
                nc.sync.dma_start(
                    aoTr[:, rbase:rbase + NUM_CORES * SL].rearrange(
                        "p (c t) -> p c t", c=NUM_CORES), a2o)
                for ts in range(SL // P):
                    for fb in range(DIM // 512):
                        pp = psA.tile([P, 512], f32, tag="mm", name="proj")
                        for dt in range(CT):
                            lo = rbase + SL * dt + P * ts
                            nc.tensor.matmul(
                                pp, aoTr[:, lo:lo + P],
                                wprojT[:, DIM * dt + 512 * fb:DIM * dt + 512 * (fb + 1)],
                                start=(dt == 0), stop=False)
                        nc.tensor.matmul(pp, ones_col[:, 0:P],
                                         bias_bf[:, 512 * fb:512 * (fb + 1)],
                                         start=False, stop=True)
                        ob = wp.tile([P, 512], f32, tag="ob", bufs=2, name="ob")
                        nc.scalar.copy(ob, pp)
                        nc.sync.dma_start(
                            out_d[SL * b + P * ts:SL * b + P * (ts + 1),
                                  512 * fb:512 * (fb + 1)], ob)

            # ---------------- main schedule ----------------
            # b0: qq0 software-pipelines b0's QKV chain (ScalarE-assisted
            # copies — exp has slack there); qq1-3 interleave b1's QKV tiles
            # (DVE copies). AV matmuls trail scores/exp by one iteration and
            # the last AV + avf copy of each q-block is carried into the next
            # block's first iteration, so the boundary never stalls the exp
            # stream. Batch-0's A2A + projection hide under b1's attention.
            LAG = 4
            state = {"pend": None, "carry": None}

            def flush_carry():
                if state["carry"] is not None:
                    state["carry"]()
                    state["carry"] = None

            def set_carry(b, qq, avp, av, et):
                def fn():
                    emit_av(b, qq, TTH - 1, av, et)
                    state["pend"] = (b, qq, emit_avf(avp))
                state["carry"] = fn

            def flush_pend():
                if state["pend"] is not None:
                    emit_norm_rest(*state["pend"])
                    state["pend"] = None

            for qq in range(N // QB):
                avp = psV.tile([65, HPC * QB], f32, tag="av", name="avp")
                av = [avp[:, QB * h:QB * (h + 1)] for h in range(HPC)]
                prev_et = None
                if qq == 0:
                    for step in range(TTH + LAG):
                        if step < TTH:
                            emit_qkv_tile(0, step, act_copies=True)
                        if step >= LAG:
                            jt = step - LAG
                            et = emit_scores_exp(0, qq, jt)
                            if jt >= 1:
                                emit_av(0, qq, jt - 1, av, prev_et)
                            prev_et = et
                else:
                    for jt in range(TTH):
                        et = emit_scores_exp(0, qq, jt)
                        if jt == 0:
                            flush_carry()
                        else:
                            emit_av(0, qq, jt - 1, av, prev_et)
                        prev_et = et
                        if jt == 2:
                            flush_pend()
                        if jt % 3 == 0:
                            nb1 = 6 * (qq - 1) + jt // 3
                            if nb1 < TTH:
                                emit_qkv_tile(1, nb1)
                set_carry(0, qq, avp, av, prev_et)
                if qq == 1:
                    for dt in range(CT):
                        nc.sync.dma_start(wprojT[:, DIM * dt:DIM * (dt + 1)],
                                          wproj_d[P * dt:P * (dt + 1), :])
            for qq in range(N // QB):
                avp = psV.tile([65, HPC * QB], f32, tag="av", name="avp")
                av = [avp[:, QB * h:QB * (h + 1)] for h in range(HPC)]
                prev_et = None
                for jt in range(TTH):
                    et = emit_scores_exp(1, qq, jt)
                    if jt == 0:
                        flush_carry()
                    else:
                        emit_av(1, qq, jt - 1, av, prev_et)
                    prev_et = et
                    if jt == 2:
                        flush_pend()
                        if qq == 0:
                            emit_a2a(0)   # all b0 chunks staged by now
                set_carry(1, qq, avp, av, prev_et)
            flush_carry()
            flush_pend()
            emit_a2a(1)
            emit_proj(0)      # runs on PE while A2A for batch 1 is in flight
            emit_proj(1)

    nc.compile()
    return nc


def _get_nc():
    if "nc" not in _CACHE:
        _CACHE["nc"] = _build_nc()
    return _CACHE["nc"]


def make_in_maps(x, Wqkv, Wproj, bproj, sin, cos):
    """Shard full (f32) inputs into per-core in_maps (pre-cast + pre-transposed)."""
    import ml_dtypes
    bf16 = ml_dtypes.bfloat16
    xT = np.ascontiguousarray(
        np.asarray(x, np.float32).reshape(T, DIM).astype(bf16).T)
    Wqkv = np.asarray(Wqkv, np.float32).astype(bf16)
    WprojT = np.ascontiguousarray(np.asarray(Wproj, np.float32).astype(bf16).T)
    bproj = np.asarray(bproj, np.float32).reshape(1, DIM)
    sin = np.asarray(sin, np.float32)
    cos = np.asarray(cos, np.float32)
    ident = np.eye(P, dtype=bf16)
    in_maps = []
    for c in range(NUM_CORES):
        r = P * c
        wq = Wqkv[r:r + P]
        wk = Wqkv[DIM + r:DIM + r + P]
        wv = Wqkv[2 * DIM + r:2 * DIM + r + P]
        in_maps.append({
            "x": xT,
            "wqkv": np.ascontiguousarray(np.concatenate([wq, wk, wv], 0).T),
            "wproj": WprojT,
            "bproj": bproj,
            "sin": sin,
            "cos": cos,
            "ident": ident,
        })
    return in_maps


def kernel(x, Wqkv, Wproj, bproj, sin, cos):
    from concourse.bass_utils import run_bass_kernel_spmd

    nc = _get_nc()
    in_maps = make_in_maps(x, Wqkv, Wproj, bproj, sin, cos)
    trace = bool(int(os.environ.get("KERNEL_TRACE", "0")))
    res = run_bass_kernel_spmd(nc, in_maps, core_ids=list(range(NUM_CORES)),
                               trace=trace)
    _CACHE["last_result"] = res
    out = np.empty((T, DIM), np.float32)
    for c in range(NUM_CORES):
        o = res.results[c]["out"]
        out[SL * c:SL * (c + 1)] = o[0:SL]
        out[N + SL * c:N + SL * (c + 1)] = o[SL:2 * SL]
    return out.reshape(B, N, DIM)


# revision 42
# speedup vs baseline: 1.0168x; 1.0168x over previous
"""Distributed multi-head attention kernel for one TRN2 chip (8 NeuronCores).

Problem: b=2, n=2048, dim=1024, heads=16, hd=64.
  qkv = x @ Wqkv.T  (qkv-major split) -> RoPE(q,k) -> softmax(q k^T/8) v
  -> merge heads -> @ Wproj.T + bproj

Sharding: each core owns 2 heads (of 16) for BOTH batches. QKV projection,
RoPE and attention are fully head-local. Two 8-way AllToAlls (one per batch,
256-token slices per core) redistribute attention outputs head-major ->
token-major; batch 0's A2A + projection hide under batch 1's attention.
Core c outputs tokens [256c:256c+256] of each batch; host reassembles.

Per-core inputs (see make_in_maps) are pre-transposed/pre-cast on the host so
no DMA-xbar transposes are needed (Tile serializes those globally):
  x        [1024, 4096] bf16  x^T: channels x flat tokens
  wqkv     [1024, 384]  bf16  (q|k|v rows for my heads)^T
  wproj    [1024, 1024] bf16  Wproj^T: [d', f]
  bproj    [1, 1024]    f32
  sin/cos  [2048, 64]   f32
  ident    [128, 128]   bf16  identity for PE transposes
  out      [512, 1024]  f32   rows 0:256 = b0 slice, 256:512 = b1 slice

All matmuls bf16 (PSUM accumulates f32). scoresT layout [k_j, q_i] (k
stationary, both heads row-packed across the 128 partitions) so softmax needs
no transposes: denominators come from a ones-column appended to v. exp on
ScalarE with fused 1/8 scale; no max subtraction (scores std ~2). The QKV
chain for each batch is software-pipelined into that batch's first
attention q-block (lag 4 tiles) so PE/ACT streams interleave.
"""

import os
import numpy as np

NUM_CORES = 8
B, N, DIM, NH, HD = 2, 2048, 1024, 16, 64
T = B * N                 # 4096 flat tokens
HPC = NH // NUM_CORES     # 2 heads per core
P = 128
CT = DIM // P             # 8 channel tiles
SL = N // NUM_CORES       # 256 output tokens per core per batch
QW = HPC * HD             # 128
FQKV = 3 * QW             # 384
QB = 512                  # attention q-block width
TTH = N // P              # 16 token tiles per batch

_CACHE = {}


def _build_nc():
    from concourse import bacc, mybir, tile

    f32 = mybir.dt.float32
    bf16 = mybir.dt.bfloat16
    Exp = mybir.ActivationFunctionType.Exp
    mult = mybir.AluOpType.mult
    add = mybir.AluOpType.add

    nc = bacc.Bacc("TRN2", target_bir_lowering=False, debug=False,
                   num_devices=NUM_CORES)

    x_d = nc.dram_tensor("x", [DIM, T], bf16, kind="ExternalInput")
    wqkv_d = nc.dram_tensor("wqkv", [DIM, FQKV], bf16, kind="ExternalInput")
    wproj_d = nc.dram_tensor("wproj", [DIM, DIM], bf16, kind="ExternalInput")
    bproj_d = nc.dram_tensor("bproj", [1, DIM], f32, kind="ExternalInput")
    sin_d = nc.dram_tensor("sin", [N, HD], f32, kind="ExternalInput")
    cos_d = nc.dram_tensor("cos", [N, HD], f32, kind="ExternalInput")
    ident_d = nc.dram_tensor("ident", [P, P], bf16, kind="ExternalInput")
    out_d = nc.dram_tensor("out", [2 * SL, DIM], f32, kind="ExternalOutput")
    a2a_in = [nc.dram_tensor(f"a2a_in{b}", [NUM_CORES * P, SL], bf16)
              for b in range(B)]
    a2a_out = [nc.dram_tensor(f"a2a_out{b}", [NUM_CORES * P, SL], bf16)
               for b in range(B)]

    with tile.TileContext(nc) as tc:
        with (
            tc.tile_pool(name="persist", bufs=1) as pers,
            tc.tile_pool(name="work", bufs=3) as wp,
            tc.tile_pool(name="expp", bufs=4) as ep,
            tc.tile_pool(name="psA", bufs=2, space="PSUM") as psA,   # qkv/bc/proj/tp
            tc.tile_pool(name="psS", bufs=2, space="PSUM") as psS,   # scores
            tc.tile_pool(name="psV", bufs=1, space="PSUM") as psV,   # av accum
        ):
            # ---------------- persistent SBUF ----------------
            wqkvT = pers.tile([P, CT * FQKV], bf16)     # ct-block: [128c, 384f]
            wprojT = pers.tile([P, CT * DIM], bf16)     # dt-block: [128d', 1024f]
            xT = pers.tile([P, CT * T], bf16)           # ct-block: [128c, 4096t]
            qT = pers.tile([P, T], bf16)                # [d(2 heads), flat t]
            kT = pers.tile([P, T], bf16)
            v_sb = pers.tile([P, HPC * (T // P) * 65], bf16)
            aoT = pers.tile([P, T], bf16)               # [d', flat t]
            aoTr = pers.tile([P, B * NUM_CORES * SL], bf16)  # per b: [d'chnk, 256t]
            sin4 = pers.tile([P, 16 * 4 * HD], bf16)
            cos4 = pers.tile([P, 16 * 4 * HD], bf16)
            sneg4 = pers.tile([P, 16 * 4 * HD], bf16)
            ones_col = pers.tile([1, P], bf16)
            bias_bf = pers.tile([1, DIM], bf16)
            ident = pers.tile([P, P], bf16)

            nc.vector.memset(ones_col, 1.0)
            nc.vector.memset(v_sb, 1.0)                 # ones cols survive

            # ---------------- prep loads (no xbar transposes) ----------------
            # scalar queue: ident, sincos, wqkv, bias (in need order);
            # sync queue: xT, first token-quarter in half-column passes so
            # tile-0's QKV matmuls start as early as possible
            nc.scalar.dma_start(ident, ident_d.ap())
            for ct in range(CT):
                nc.scalar.dma_start(wqkvT[:, FQKV * ct:FQKV * (ct + 1)],
                                    wqkv_d[P * ct:P * (ct + 1), :])
            sin_f = wp.tile([P, 16 * HD], f32, tag="scload", bufs=1)
            cos_f = wp.tile([P, 16 * HD], f32, tag="scload2", bufs=1)
            nc.scalar.dma_start(
                sin_f.rearrange("p (pt d) -> p pt d", pt=16),
                sin_d.ap().rearrange("(pt p) d -> p pt d", p=P))
            nc.scalar.dma_start(
                cos_f.rearrange("p (pt d) -> p pt d", pt=16),
                cos_d.ap().rearrange("(pt p) d -> p pt d", p=P))
            for half in range(2):
                for ct in range(CT):
                    nc.sync.dma_start(
                        xT[:, T * ct + 512 * half:T * ct + 512 * (half + 1)],
                        x_d[P * ct:P * (ct + 1), 512 * half:512 * (half + 1)])
            for tq in range(1, 4):
                for ct in range(CT):
                    nc.sync.dma_start(
                        xT[:, T * ct + 1024 * tq:T * ct + 1024 * (tq + 1)],
                        x_d[P * ct:P * (ct + 1), 1024 * tq:1024 * (tq + 1)])

            bt = wp.tile([1, DIM], f32, tag="bload", bufs=1)
            nc.scalar.dma_start(bt, bproj_d[:, :])
            nc.vector.tensor_copy(bias_bf, bt)
            s4 = sin4.rearrange("p (pt c d) -> p pt c d", pt=16, c=4)
            c4 = cos4.rearrange("p (pt c d) -> p pt c d", pt=16, c=4)
            n4 = sneg4.rearrange("p (pt c d) -> p pt c d", pt=16, c=4)
            sf = sin_f.rearrange("p (pt d) -> p pt d", pt=16)
            cf = cos_f.rearrange("p (pt d) -> p pt d", pt=16)
            for c in range(4):
                nc.vector.tensor_copy(s4[:, :, c, :], sf)
                nc.vector.tensor_copy(c4[:, :, c, :], cf)
            nc.vector.tensor_scalar_mul(n4[:, :, :, 0:32], s4[:, :, :, 0:32], -1.0)
            nc.vector.tensor_copy(n4[:, :, :, 32:64], s4[:, :, :, 32:64])

            def emit_qkv_tile(b, tt, act_copies=False):
                """QKV matmul + RoPE + PE transposes for one 128-token tile.

                act_copies: route PSUM->SBUF copies to ScalarE (only safe in
                windows where the exp stream has slack, i.e. b0's first
                q-block)."""
                cp = nc.scalar.copy if act_copies else nc.vector.tensor_copy
                ftt = TTH * b + tt
                qkvp = psA.tile([P, 512], f32, tag="mm", name="qkvp")
                for ct in range(CT):
                    base = T * ct + N * b
                    nc.tensor.matmul(
                        qkvp[:, 0:FQKV],
                        xT[:, base + P * tt:base + P * (tt + 1)],
                        wqkvT[:, FQKV * ct:FQKV * (ct + 1)],
                        start=(ct == 0), stop=(ct == CT - 1))
                qkc = wp.tile([P, 2 * QW], bf16, tag="qkc")
                cp(qkc, qkvp[:, 0:2 * QW])
                pt = tt % 16
                qk3 = qkc.rearrange("p (c d) -> p c d", c=4)
                t1 = wp.tile([P, 2 * QW], bf16, tag="t1")
                t13 = t1.rearrange("p (c d) -> p c d", c=4)
                nc.vector.tensor_tensor(t13[:, :, 0:32], qk3[:, :, 32:64],
                                        n4[:, pt, :, 0:32], mult)
                nc.vector.tensor_tensor(t13[:, :, 32:64], qk3[:, :, 0:32],
                                        n4[:, pt, :, 32:64], mult)
                qkcos = wp.tile([P, 2 * QW], bf16, tag="qkcos")
                nc.vector.tensor_tensor(
                    qkcos, qkc, cos4[:, 4 * HD * pt:4 * HD * (pt + 1)], mult)
                qrope = wp.tile([P, QW], bf16, tag="qrope")
                krope = wp.tile([P, QW], bf16, tag="krope")
                nc.vector.tensor_tensor(qrope, qkcos[:, 0:QW], t1[:, 0:QW], add)
                nc.vector.tensor_tensor(krope, qkcos[:, QW:2 * QW],
                                        t1[:, QW:2 * QW], add)
                tp = psA.tile([P, 2 * P], bf16, tag="mm", name="tp")
                nc.tensor.transpose(tp[:, 0:P], qrope, ident)
                nc.tensor.transpose(tp[:, P:2 * P], krope, ident)
                cp(qT[:, P * ftt:P * (ftt + 1)], tp[:, 0:P])
                cp(kT[:, P * ftt:P * (ftt + 1)], tp[:, P:2 * P])
                vv = v_sb.rearrange("p (h t e) -> p h t e", h=HPC, t=T // P)
                nc.vector.tensor_copy(
                    vv[:, :, ftt, 0:HD],
                    qkvp[:, 2 * QW:3 * QW].rearrange("p (h d) -> p h d", h=HPC))

            def emit_scores_exp(b, qq, jt):
                """Scores + exp for one (q-block, j-tile); returns the exp tile."""
                ftt = TTH * b + jt
                sp = psS.tile([P, HPC * QB], f32, tag="scores", name="sp")
                for h in range(HPC):
                    nc.tensor.matmul(
                        sp[:, QB * h:QB * (h + 1)],
                        kT[HD * h:HD * (h + 1), P * ftt:P * (ftt + 1)],
                        qT[HD * h:HD * (h + 1),
                           N * b + QB * qq:N * b + QB * (qq + 1)],
                        start=True, stop=True)
                et = ep.tile([P, HPC * QB], bf16, tag="expT", name="et")
                nc.scalar.activation(et, sp, Exp, scale=float(HD) ** -0.5)
                return et

            def emit_av(b, qq, jt, av, et):
                ftt = TTH * b + jt
                for h in range(HPC):
                    blk = (h * (T // P) + ftt) * 65
                    nc.tensor.matmul(av[h], v_sb[:, blk:blk + 65],
                                     et[:, QB * h:QB * (h + 1)],
                                     start=(jt == 0), stop=(jt == TTH - 1))

            def emit_avf(avp):
                avf = wp.tile([65, HPC * QB], f32, tag="avf", bufs=2, name="avf")
                nc.vector.tensor_copy(avf, avp)
                return avf

            def emit_norm_rest(b, qq, avf):
                """Denominator broadcast + reciprocal + normalize, plus the A2A
                staging of this q-block's two 256-token chunks."""
                for h in range(HPC):
                    sums = wp.tile([1, QB], bf16, tag="sums", name="sums")
                    nc.vector.tensor_copy(sums, avf[64:65, QB * h:QB * (h + 1)])
                    bc = psA.tile([64, QB], f32, tag="mm", name="bc")
                    nc.tensor.matmul(bc, ones_col[:, 0:64], sums,
                                     start=True, stop=True)
                    rc = wp.tile([64, QB], f32, tag="recip", bufs=2, name="rc")
                    nc.vector.reciprocal_approx_fast(rc, bc)
                    nc.vector.tensor_tensor(
                        aoT[HD * h:HD * (h + 1),
                            N * b + QB * qq:N * b + QB * (qq + 1)],
                        avf[0:64, QB * h:QB * (h + 1)], rc, mult)
                # stage chunks 2qq, 2qq+1 of this batch's A2A input
                r0 = 2 * P * qq
                a2i = a2a_in[b][r0:r0 + 2 * P].rearrange("(c p) t -> p c t", p=P)
                nc.sync.dma_start(
                    a2i, aoT[:, N * b + QB * qq:N * b + QB * (qq + 1)].rearrange(
                        "p (c t) -> p c t", c=2))

            def emit_a2a(b):
                nc.gpsimd.collective_compute(
                    "AllToAll", mybir.AluOpType.bypass,
                    replica_groups=[list(range(NUM_CORES))],
                    ins=[a2a_in[b].ap().opt()], outs=[a2a_out[b].ap().opt()])

            def emit_proj(b):
                """Fetch A2A result for batch b and project my 256-token slice."""
                rbase = NUM_CORES * SL * b
                a2o = a2a_out[b].ap().rearrange("(c p) t -> p c t", p=P)
                nc.sync.dma_start(
                    aoTr[:, rbase:rbase + NUM_CORES * SL].rearrange(
                        "p (c t) -> p c t", c=NUM_CORES), a2o)
                for ts in range(SL // P):
                    for fb in range(DIM // 512):
                        pp = psA.tile([P, 512], f32, tag="mm", name="proj")
                        for dt in range(CT):
                            lo = rbase + SL * dt + P * ts
                            nc.tensor.matmul(
                                pp, aoTr[:, lo:lo + P],
                                wprojT[:, DIM * dt + 512 * fb:DIM * dt + 512 * (fb + 1)],
                                start=(dt == 0), stop=False)
                        nc.tensor.matmul(pp, ones_col[:, 0:P],
                                         bias_bf[:, 512 * fb:512 * (fb + 1)],
                                         start=False, stop=True)
                        ob = wp.tile([P, 512], f32, tag="ob", bufs=2, name="ob")
                        nc.scalar.copy(ob, pp)
                        nc.sync.dma_start(
                            out_d[SL * b + P * ts:SL * b + P * (ts + 1),
                                  512 * fb:512 * (fb + 1)], ob)

            # ---------------- main schedule ----------------
            # b0: qq0 software-pipelines b0's QKV chain (ScalarE-assisted
            # copies — exp has slack there); qq1-3 interleave b1's QKV tiles
            # (DVE copies). AV matmuls trail scores/exp by one iteration and
            # the last AV + avf copy of each q-block is carried into the next
            # block's first iteration, so the boundary never stalls the exp
            # stream. Batch-0's A2A + projection hide under b1's attention.
            LAG = 4
            state = {"pend": None, "carry": None}

            def flush_carry():
                if state["carry"] is not None:
                    state["carry"]()
                    state["carry"] = None

            def set_carry(b, qq, avp, av, et):
                def fn():
                    emit_av(b, qq, TTH - 1, av, et)
                    state["pend"] = (b, qq, emit_avf(avp))
                state["carry"] = fn

            def flush_pend():
                if state["pend"] is not None:
                    emit_norm_rest(*state["pend"])
                    state["pend"] = None

            for qq in range(N // QB):
                avp = psV.tile([65, HPC * QB], f32, tag="av", name="avp")
                av = [avp[:, QB * h:QB * (h + 1)] for h in range(HPC)]
                prev_et = None
                if qq == 0:
                    for step in range(TTH + LAG):
                        if step < TTH:
                            emit_qkv_tile(0, step, act_copies=True)
                        if step >= LAG:
                            jt = step - LAG
                            et = emit_scores_exp(0, qq, jt)
                            if jt >= 1:
                                emit_av(0, qq, jt - 1, av, prev_et)
                            prev_et = et
                else:
                    for jt in range(TTH):
                        et = emit_scores_exp(0, qq, jt)
                        if jt == 0:
                            flush_carry()
                        else:
                            emit_av(0, qq, jt - 1, av, prev_et)
                        prev_et = et
                        if jt == 2:
                            flush_pend()
                        if jt % 3 == 0:
                            nb1 = 6 * (qq - 1) + jt // 3
                            if nb1 < TTH:
                                emit_qkv_tile(1, nb1)
                set_carry(0, qq, avp, av, prev_et)
                if qq == 1:
                    for dt in range(CT):
                        nc.sync.dma_start(wprojT[:, DIM * dt:DIM * (dt + 1)],
                                          wproj_d[P * dt:P * (dt + 1), :])
            for qq in range(N // QB):
                avp = psV.tile([65, HPC * QB], f32, tag="av", name="avp")
                av = [avp[:, QB * h:QB * (h + 1)] for h in range(HPC)]
                prev_et = None
                for jt in range(TTH):
                    et = emit_scores_exp(1, qq, jt)
                    if jt == 0:
                        flush_carry()
                    else:
                        emit_av(1, qq, jt - 1, av, prev_et)
                    prev_et = et
                    if jt == 2:
                        flush_pend()
                        if qq == 0:
                            emit_a2a(0)   # all b0 chunks staged by now
                set_carry(1, qq, avp, av, prev_et)
            flush_carry()
            flush_pend()
            emit_a2a(1)
            emit_proj(0)      # runs on PE while A2A for batch 1 is in flight
            emit_proj(1)

    nc.compile()
    return nc


def _get_nc():
    if "nc" not in _CACHE:
        _CACHE["nc"] = _build_nc()
    return _CACHE["nc"]


def make_in_maps(x, Wqkv, Wproj, bproj, sin, cos):
    """Shard full (f32) inputs into per-core in_maps (pre-cast + pre-transposed)."""
    import ml_dtypes
    bf16 = ml_dtypes.bfloat16
    xT = np.ascontiguousarray(
        np.asarray(x, np.float32).reshape(T, DIM).astype(bf16).T)
    Wqkv = np.asarray(Wqkv, np.float32).astype(bf16)
    WprojT = np.ascontiguousarray(np.asarray(Wproj, np.float32).astype(bf16).T)
    bproj = np.asarray(bproj, np.float32).reshape(1, DIM)
    sin = np.asarray(sin, np.float32)
    cos = np.asarray(cos, np.float32)
    ident = np.eye(P, dtype=bf16)
    in_maps = []
    for c in range(NUM_CORES):
        r = P * c
        wq = Wqkv[r:r + P]
        wk = Wqkv[DIM + r:DIM + r + P]
        wv = Wqkv[2 * DIM + r:2 * DIM + r + P]
        in_maps.append({
            "x": xT,
            "wqkv": np.ascontiguousarray(np.concatenate([wq, wk, wv], 0).T),
            "wproj": WprojT,
            "bproj": bproj,
            "sin": sin,
            "cos": cos,
            "ident": ident,
        })
    return in_maps


def kernel(x, Wqkv, Wproj, bproj, sin, cos):
    from concourse.bass_utils import run_bass_kernel_spmd

    nc = _get_nc()
    in_maps = make_in_maps(x, Wqkv, Wproj, bproj, sin, cos)
    trace = bool(int(os.environ.get("KERNEL_TRACE", "0")))
    res = run_bass_kernel_spmd(nc, in_maps, core_ids=list(range(NUM_CORES)),
                               trace=trace)
    _CACHE["last_result"] = res
    out = np.empty((T, DIM), np.float32)
    for c in range(NUM_CORES):
        o = res.results[c]["out"]
        out[SL * c:SL * (c + 1)] = o[0:SL]
        out[N + SL * c:N + SL * (c + 1)] = o[SL:2 * SL]
    return out.reshape(B, N, DIM)
